# revision 2
# baseline (speedup 1.0000x reference)
"""Trainium2 Bass kernel for nn_DihedralGroupConv.

Math: reference computes
    filt[c,i,d,o] = sum_g perm[g,i,o] * weight[g,c,d]
    out = x.reshape(B,-1) @ filt.reshape(C*2n, D*2n)
i.e. out[b,d,o] = sum_{g,c} weight[g,c,d] * x[b,c, idx_g(o)]
where perm[g] are permutation matrices of the dihedral regular
representation: each is a half-wise cyclic shift of either x itself
(rotations) or of the reflected array xr (reflections).

Kernel strategy (data-parallel over batch, 64 b per core):
  - Host precomputes xr and halo-padded (216 = 200 + 2*8) per-half images
    of x and xr, laid out directly as SBUF images AX/AR[128, 16, 2, 216]
    with partition = 32*(b%4) + c, so each generator contribution over a
    quad of 4 batch elements is ONE full-width contiguous-window matmul.
  - Weights are packed 4-way block-diagonally [128,128] (one 32x32
    weight block per batch element of the quad). Per quad, one matmul
    per generator (K=128, M=128, N=400, float32r fast mode) accumulates
    all generators into one PSUM bank.
  - DVE copies PSUM->SBUF staging, DMA writes a scrambled [128, 6400]
    output image which the host unscrambles.
All DMAs are pure 128-partition contiguous-run transfers.
"""

import numpy as np

import concourse.bass as bass  # noqa: F401  (kept for users of this module)
import concourse.mybir as mybir
from concourse import bacc
from concourse.tile import TileContext
from concourse.bass_utils import run_bass_kernel_spmd

# Problem constants (hardcoded per harness contract).
B = 512
C = 32          # in channels
D = 32          # out channels
N = 200         # half length; 2N = 400
L = 2 * N
N_CORES = 8
BPC = B // N_CORES          # 64 batch per core
NQ = BPC // 4               # 16 quads (4 batch / quad)
HALO = 8
PH = N + 2 * HALO           # 216 padded half length

_DT_IN = mybir.dt.float32r  # fp32 data, fast PE mode (1 cyc/col @ N>=256)
_DT_OUT = mybir.dt.float32

_cache = {}


def _derive_gens(perm):
    """Classify each generator as (is_refl, shift s) with y[o] = base[(o+s)%N]
    per half, where base is x (rotation) or xr (reflection)."""
    n = N
    o = np.arange(n)
    gens = []
    for g in range(perm.shape[0]):
        idx = np.argmax(perm[g], axis=0).astype(np.int64)  # y[o] = x[idx[o]]
        # rotation candidate: idx[o] = (o - r) % n ; idx[n+o] = n + (o-r)%n
        r = int((-idx[0]) % n)
        rot = np.concatenate([(o - r) % n, n + (o - r) % n])
        if np.array_equal(idx, rot):
            s = -r if r <= n // 2 else n - r
            gens.append((False, s))
            continue
        # reflection candidate: y[o] = xr[(o+r)%n per half] with
        # xr[t] = x[n + (-t)%n], xr[n+t] = x[(-t)%n]
        # => idx[o] = n + (-o-r)%n ; idx[n+o] = (-o-r)%n
        r = int(idx[0] - n) % n     # idx[0] = n + (-r)%n -> (-r)%n
        r = (-r) % n
        refl = np.concatenate([n + (-o - r) % n, (-o - r) % n])
        if np.array_equal(idx, refl):
            s = r if r <= n // 2 else r - n
            gens.append((True, s))
            continue
        raise NotImplementedError(f"perm[{g}] is not a dihedral rep matrix")
    for is_refl, s in gens:
        if not (-HALO <= s <= HALO):
            raise NotImplementedError(f"shift {s} exceeds halo {HALO}")
    return gens


def _build_program(gens):
    """Build + compile the SPMD Bass program (identical on all cores)."""
    rot = [(j, s) for j, (is_r, s) in enumerate(gens) if not is_r]
    refl = [(j, s) for j, (is_r, s) in enumerate(gens) if is_r]
    nblk = len(rot) + len(refl)

    nc = bacc.Bacc("TRN2", target_bir_lowering=False, debug=False,
                   num_devices=N_CORES)
    ax_d = nc.dram_tensor("ax", [128, NQ, 2, PH], _DT_IN,
                          kind="ExternalInput")
    ar_d = (nc.dram_tensor("ar", [128, NQ, 2, PH], _DT_IN,
                           kind="ExternalInput") if refl else None)
    ws_d = nc.dram_tensor("ws", [128, 128 * nblk], _DT_IN,
                          kind="ExternalInput")
    outr_d = nc.dram_tensor("outr", [128, NQ * L], _DT_OUT,
                            kind="ExternalOutput")

    CHUNK_Q = 4  # DMA granularity along quads
    with TileContext(nc) as tc:
        with (
            tc.tile_pool(name="arrp", bufs=1) as arrp,
            tc.tile_pool(name="wsp", bufs=1) as wsp,
            tc.tile_pool(name="stg", bufs=4) as stgp,
            tc.tile_pool(name="psum", bufs=4, space="PSUM") as psump,
        ):
            ws_sb = wsp.tile([128, 128 * nblk], _DT_IN)
            nc.sync.dma_start(out=ws_sb[:, :], in_=ws_d[:, :])
            ax_sb = arrp.tile([128, NQ, 2, PH], _DT_IN, name="ax_sb")
            ar_sb = (arrp.tile([128, NQ, 2, PH], _DT_IN, name="ar_sb")
                     if refl else None)
            # interleave the x / xr chunk loads so early quads unblock asap
            for c0 in range(0, NQ, CHUNK_Q):
                nc.sync.dma_start(out=ax_sb[:, c0:c0 + CHUNK_Q],
                                  in_=ax_d[:, c0:c0 + CHUNK_Q])
                if refl:
                    nc.sync.dma_start(out=ar_sb[:, c0:c0 + CHUNK_Q],
                                      in_=ar_d[:, c0:c0 + CHUNK_Q])

            # per quad: one matmul per generator, all accumulating in PSUM
            mm_descs = []  # (src_is_refl, weight block col, window start)
            for k, (_, s) in enumerate(rot):
                mm_descs.append((False, 128 * k, s + HALO))
            for k, (_, s) in enumerate(refl):
                mm_descs.append((True, 128 * (len(rot) + k), s + HALO))

            for q in range(NQ):
                ps = psump.tile([128, L], mybir.dt.float32)
                for i, (is_r, wc, u0) in enumerate(mm_descs):
                    src = ar_sb if is_r else ax_sb
                    nc.tensor.matmul(
                        ps[:, :],
                        ws_sb[:, wc:wc + 128],
                        src[:, q, :, u0:u0 + N],
                        start=(i == 0), stop=(i == len(mm_descs) - 1),
                    )
                stg = stgp.tile([128, L], _DT_OUT)
                nc.vector.tensor_copy(out=stg[:, :], in_=ps[:, :])
                nc.sync.dma_start(out=outr_d[:, q * L:(q + 1) * L],
                                  in_=stg[:, :])
    nc.compile()
    return nc


def _host_images(x, weight, gens):
    """Build per-core AX/AR images and the packed block-diag weight image."""
    n = N
    t = np.arange(n)
    refl_idx = np.concatenate([n + (-t) % n, (-t) % n])

    rot = [(j, s) for j, (is_r, s) in enumerate(gens) if not is_r]
    refl = [(j, s) for j, (is_r, s) in enumerate(gens) if is_r]
    nblk = len(rot) + len(refl)

    pad_idx = (np.arange(PH) - HALO) % n
    xh = x.reshape(B, C, 2, n)[:, :, :, pad_idx]          # [B, C, 2, PH]
    xrh = (x[:, :, refl_idx].reshape(B, C, 2, n)[:, :, :, pad_idx]
           if refl else None)

    ws = np.zeros((128, 128 * nblk), dtype=np.float32)
    for k, (j, _) in enumerate(rot + refl):
        for u in range(4):
            ws[32 * u:32 * (u + 1),
               128 * k + 32 * u:128 * k + 32 * (u + 1)] = weight[j]

    def img(a, core):
        sl = a[core * BPC:(core + 1) * BPC]               # [64, C, 2, PH]
        out = np.empty((128, NQ, 2, PH), dtype=np.float32)
        for u in range(4):
            out[32 * u:32 * (u + 1)] = sl[u::4].transpose(1, 0, 2, 3)
        return np.ascontiguousarray(out)

    axs = [img(xh, c) for c in range(N_CORES)]
    ars = [img(xrh, c) for c in range(N_CORES)] if refl else None
    return axs, ars, ws


def _unscramble(outr):
    """outr[32*(b%4)+d, (b>>2)*L + o] -> out shard [BPC, D, L]."""
    r = outr.reshape(4, D, NQ, L)           # [b%4, d, q, o]
    r = r.transpose(2, 0, 1, 3)             # [q, b%4, d, o]
    return np.ascontiguousarray(r.reshape(BPC, D, L))


def kernel(x, weight, perm, _trace=False):
    x = np.asarray(x, dtype=np.float32)
    weight = np.asarray(weight, dtype=np.float32)
    perm = np.asarray(perm, dtype=np.float32)

    gens = _derive_gens(perm)
    key = tuple(gens)
    if key not in _cache:
        _cache[key] = _build_program(gens)
    nc = _cache[key]

    axs, ars, ws = _host_images(x, weight, gens)
    in_maps = []
    for c in range(N_CORES):
        m = {"ax": axs[c], "ws": ws}
        if ars is not None:
            m["ar"] = ars[c]
        in_maps.append(m)
    res = run_bass_kernel_spmd(nc, in_maps, core_ids=list(range(N_CORES)),
                               trace=_trace)
    out = np.concatenate([_unscramble(res.results[c]["outr"])
                          for c in range(N_CORES)], axis=0)
    if _trace:
        kernel.last_exec_time_ns = res.exec_time_ns
        kernel.last_results = res
    return out


# revision 19
# speedup vs baseline: 1.6216x; 1.6216x over previous
"""Trainium2 Bass kernel for nn_DihedralGroupConv.

Math: reference computes
    filt[c,i,d,o] = sum_g perm[g,i,o] * weight[g,c,d]
    out = x.reshape(B,-1) @ filt.reshape(C*2n, D*2n)
i.e. out[b,d,o] = sum_{g,c} weight[g,c,d] * x[b,c, idx_g(o)]
where perm[g] are permutation matrices of the dihedral regular
representation: each is a half-wise cyclic shift of either x itself
(rotations) or of the reflected array xr (reflections).

Kernel strategy (data-parallel over batch, 64 b per core):
  - Host precomputes a halo-padded (216 = 200 + 2*8) per-half image of x,
    laid out directly as the SBUF image AX[128, 16, 2, 216] with
    partition = 32*(b%4) + c, so each generator contribution over a quad
    of 4 batch elements is ONE contiguous-window matmul: rotations read a
    shifted window forward, reflections read a shifted window BACKWARD
    (negative-stride access pattern with swapped halves) -- no second
    image needed.
  - Per quad and generator, four independent 32x32 PE-quadrant matmuls
    (batch element u: SBUF rows 32u -> PSUM partitions 32u) run
    concurrently in the array; the tiny 32-column weight loads hide
    under the other quadrants' matmuls. All generators accumulate into
    one PSUM bank per quad; quads are processed in ping-pong pairs so
    one quad's matmul stream hides the other's PSUM drain latency.
  - DVE/ACT copy PSUM->SBUF staging (cast to fp16), DMA writes a
    scrambled [128, 6400] output image which the host unscrambles.
All DMAs are pure 128-partition contiguous-run transfers.
"""

import numpy as np

import concourse.bass as bass  # noqa: F401  (kept for users of this module)
import concourse.mybir as mybir
from concourse import bacc
from concourse.tile import TileContext
from concourse.bass_utils import run_bass_kernel_spmd

# Problem constants (hardcoded per harness contract).
B = 512
C = 32          # in channels
D = 32          # out channels
N = 200         # half length; 2N = 400
L = 2 * N
N_CORES = 8
BPC = B // N_CORES          # 64 batch per core
NQ = BPC // 4               # 16 quads (4 batch / quad)
HALO = 8
PH = N + 2 * HALO           # 216 padded half length

_DT_IN = mybir.dt.float16   # 1 cyc/col PE mode, 1-pass weight load, half DMA
_DT_OUT = mybir.dt.float16  # output staged/stored as fp16, host casts to f32
_NP_IN = np.float16

_cache = {}


def _derive_gens(perm):
    """Classify each generator as (is_refl, shift s) with y[o] = base[(o+s)%N]
    per half, where base is x (rotation) or xr (reflection)."""
    n = N
    o = np.arange(n)
    gens = []
    for g in range(perm.shape[0]):
        idx = np.argmax(perm[g], axis=0).astype(np.int64)  # y[o] = x[idx[o]]
        # rotation candidate: idx[o] = (o - r) % n ; idx[n+o] = n + (o-r)%n
        r = int((-idx[0]) % n)
        rot = np.concatenate([(o - r) % n, n + (o - r) % n])
        if np.array_equal(idx, rot):
            s = -r if r <= n // 2 else n - r
            gens.append((False, s))
            continue
        # reflection candidate: y[o] = xr[(o+r)%n per half] with
        # xr[t] = x[n + (-t)%n], xr[n+t] = x[(-t)%n]
        # => idx[o] = n + (-o-r)%n ; idx[n+o] = (-o-r)%n
        r = int(idx[0] - n) % n     # idx[0] = n + (-r)%n -> (-r)%n
        r = (-r) % n
        refl = np.concatenate([n + (-o - r) % n, (-o - r) % n])
        if np.array_equal(idx, refl):
            s = r if r <= n // 2 else r - n
            gens.append((True, s))
            continue
        raise NotImplementedError(f"perm[{g}] is not a dihedral rep matrix")
    for is_refl, s in gens:
        if is_refl:
            ok = -(HALO - 1) <= s <= HALO
        else:
            ok = -HALO <= s <= HALO
        if not ok:
            raise NotImplementedError(f"shift {s} exceeds halo {HALO}")
    return gens


def _build_program(gens):
    """Build + compile the SPMD Bass program (identical on all cores)."""
    rot = [(j, s) for j, (is_r, s) in enumerate(gens) if not is_r]
    refl = [(j, s) for j, (is_r, s) in enumerate(gens) if is_r]
    nblk = len(rot) + len(refl)

    nc = bacc.Bacc("TRN2", target_bir_lowering=False, debug=False,
                   num_devices=N_CORES)
    ax_d = nc.dram_tensor("ax", [128, NQ, 2, PH], _DT_IN,
                          kind="ExternalInput")
    ws_d = nc.dram_tensor("ws", [128, 32 * nblk], _DT_IN,
                          kind="ExternalInput")
    outr_d = nc.dram_tensor("outr", [128, NQ * L], _DT_OUT,
                            kind="ExternalOutput")

    CHUNKS = [2, 4, 4, 6]   # DMA chunk sizes along quads (small first
                            # chunk -> matmuls start earlier)
    with TileContext(nc) as tc:
        with (
            tc.tile_pool(name="arrp", bufs=1) as arrp,
            tc.tile_pool(name="wsp", bufs=1) as wsp,
            tc.tile_pool(name="stg", bufs=1) as stgp,
            tc.tile_pool(name="psum", bufs=1, space="PSUM") as psump,
        ):
            ws_sb = wsp.tile([128, 32 * nblk], _DT_IN)
            nc.sync.dma_start(out=ws_sb[:, :], in_=ws_d[:, :])
            ax_sb = arrp.tile([128, NQ, 2, PH], _DT_IN, name="ax_sb")
            c0 = 0
            for cq in CHUNKS:
                nc.sync.dma_start(out=ax_sb[:, c0:c0 + cq],
                                  in_=ax_d[:, c0:c0 + cq])
                c0 += cq

            # per quad: one matmul per generator per 32x32 PE quadrant
            # (batch element u -> SBUF rows 32u, PSUM partitions 32u);
            # the 4 quadrants run concurrently in the array.
            # (is_refl, weight block col, window param)
            mm_descs = []
            for k, (_, s) in enumerate(rot):
                mm_descs.append((False, 32 * k, s + HALO))
            for k, (_, s) in enumerate(refl):
                mm_descs.append((True, 32 * (len(rot) + k), s))
            ng = len(mm_descs)

            axt = ax_sb[:, :, :, :]
            pstride = axt.ap[0][0]      # free elems per partition

            def rhs_ap(p0, q, is_r, w):
                if not is_r:
                    return ax_sb[p0:p0 + 32, q, :, w:w + N]
                # reflection: swapped halves, backward o scan;
                # out (h, o) reads src[1-h, (HALO+200) - o - s]
                off = p0 * pstride + q * (2 * PH) + PH + (PH - HALO - w)
                return bass.AP(axt.tensor, off,
                               [[pstride, 32], [-PH, 2], [-1, N]])

            # pre-allocated rotating tiles (fewer tile instances -> fewer
            # semaphores -> shorter kernel-tail sem-reset storm)
            pstiles = [psump.tile([128, L], mybir.dt.float32,
                                  name=f"ps{i}") for i in range(4)]
            stgs = [stgp.tile([128, 2, L], _DT_OUT, name=f"stg{i}")
                    for i in range(4)]

            # ping-pong pairs of quads: quad B's matmul stream hides
            # quad A's PSUM drain between accumulation groups
            for qp in range(NQ // 2):
                pair = (2 * qp, 2 * qp + 1)
                pss = [pstiles[(2 * qp) % 4], pstiles[(2 * qp + 1) % 4]]
                for i, (is_r, wc, w) in enumerate(mm_descs):
                    for u in range(4):
                        p0 = 32 * u
                        for pi, q in enumerate(pair):
                            nc.tensor.matmul(
                                pss[pi][p0:p0 + 32, :],
                                ws_sb[p0:p0 + 32, wc:wc + 32],
                                rhs_ap(p0, q, is_r, w),
                                start=(i == 0), stop=(i == ng - 1),
                                tile_position=(p0, p0),
                            )
                stg = stgs[qp % 4]
                for pi, q in enumerate(pair):
                    if pi == 0:
                        nc.vector.tensor_copy(out=stg[:, 0], in_=pss[pi][:, :])
                    else:
                        nc.scalar.copy(out=stg[:, 1], in_=pss[pi][:, :])
                nc.sync.dma_start(out=outr_d[:, 2 * qp * L:(2 * qp + 2) * L],
                                  in_=stg[:, :, :])
    nc.compile()
    return nc


def _host_images(x, weight, gens):
    """Build per-core AX images and the packed weight image."""
    n = N
    rot = [(j, s) for j, (is_r, s) in enumerate(gens) if not is_r]
    refl = [(j, s) for j, (is_r, s) in enumerate(gens) if is_r]
    nblk = len(rot) + len(refl)

    pad_idx = (np.arange(PH) - HALO) % n
    xh = x.reshape(B, C, 2, n)[:, :, :, pad_idx]          # [B, C, 2, PH]

    ws = np.zeros((128, 32 * nblk), dtype=_NP_IN)
    for k, (j, _) in enumerate(rot + refl):
        for u in range(4):
            ws[32 * u:32 * (u + 1), 32 * k:32 * (k + 1)] = weight[j]

    def img(a, core):
        sl = a[core * BPC:(core + 1) * BPC]               # [64, C, 2, PH]
        out = np.empty((128, NQ, 2, PH), dtype=_NP_IN)
        for u in range(4):
            out[32 * u:32 * (u + 1)] = sl[u::4].transpose(1, 0, 2, 3)
        return np.ascontiguousarray(out)

    axs = [img(xh, c) for c in range(N_CORES)]
    return axs, ws


def _unscramble(outr):
    """outr[32*(b%4)+d, (b>>2)*L + o] -> out shard [BPC, D, L]."""
    r = outr.astype(np.float32).reshape(4, D, NQ, L)    # [b%4, d, q, o]
    r = r.transpose(2, 0, 1, 3)                         # [q, b%4, d, o]
    return np.ascontiguousarray(r.reshape(BPC, D, L))


def kernel(x, weight, perm, _trace=False):
    x = np.asarray(x, dtype=np.float32)
    weight = np.asarray(weight, dtype=np.float32)
    perm = np.asarray(perm, dtype=np.float32)

    gens = _derive_gens(perm)
    key = tuple(gens)
    if key not in _cache:
        _cache[key] = _build_program(gens)
    nc = _cache[key]

    axs, ws = _host_images(x, weight, gens)
    in_maps = [{"ax": axs[c], "ws": ws} for c in range(N_CORES)]
    res = run_bass_kernel_spmd(nc, in_maps, core_ids=list(range(N_CORES)),
                               trace=_trace)
    out = np.concatenate([_unscramble(res.results[c]["outr"])
                          for c in range(N_CORES)], axis=0)
    if _trace:
        kernel.last_exec_time_ns = res.exec_time_ns
        kernel.last_results = res
    return out


# revision 20
# speedup vs baseline: 1.6725x; 1.0314x over previous
"""Trainium2 Bass kernel for nn_DihedralGroupConv.

Math: reference computes
    filt[c,i,d,o] = sum_g perm[g,i,o] * weight[g,c,d]
    out = x.reshape(B,-1) @ filt.reshape(C*2n, D*2n)
i.e. out[b,d,o] = sum_{g,c} weight[g,c,d] * x[b,c, idx_g(o)]
where perm[g] are permutation matrices of the dihedral regular
representation: each is a half-wise cyclic shift of either x itself
(rotations) or of the reflected array xr (reflections).

Kernel strategy (data-parallel over batch, 64 b per core):
  - Host precomputes a halo-padded (216 = 200 + 2*8) per-half image of x,
    laid out directly as the SBUF image AX[128, 16, 2, 216] with
    partition = 32*(b%4) + c, so each generator contribution over a quad
    of 4 batch elements is ONE contiguous-window matmul: rotations read a
    shifted window forward, reflections read a shifted window BACKWARD
    (negative-stride access pattern with swapped halves) -- no second
    image needed.
  - Per quad and generator, four independent 32x32 PE-quadrant matmuls
    (batch element u: SBUF rows 32u -> PSUM partitions 32u) run
    concurrently in the array; the tiny 32-column weight loads hide
    under the other quadrants' matmuls. All generators accumulate into
    one PSUM bank per quad; quads are processed in ping-pong pairs so
    one quad's matmul stream hides the other's PSUM drain latency.
  - DVE/ACT copy PSUM->SBUF staging (cast to fp16), DMA writes a
    scrambled [128, 6400] output image which the host unscrambles.
All DMAs are pure 128-partition contiguous-run transfers.
"""

import numpy as np

import concourse.bass as bass  # noqa: F401  (kept for users of this module)
import concourse.mybir as mybir
from concourse import bacc
from concourse.tile import TileContext
from concourse.bass_utils import run_bass_kernel_spmd

# Problem constants (hardcoded per harness contract).
B = 512
C = 32          # in channels
D = 32          # out channels
N = 200         # half length; 2N = 400
L = 2 * N
N_CORES = 8
BPC = B // N_CORES          # 64 batch per core
NQ = BPC // 4               # 16 quads (4 batch / quad)
HALO = 8
PH = N + 2 * HALO           # 216 padded half length

_DT_IN = mybir.dt.float16   # 1 cyc/col PE mode, 1-pass weight load, half DMA
_DT_OUT = mybir.dt.float16  # output staged/stored as fp16, host casts to f32
_NP_IN = np.float16

_cache = {}


def _derive_gens(perm):
    """Classify each generator as (is_refl, shift s) with y[o] = base[(o+s)%N]
    per half, where base is x (rotation) or xr (reflection)."""
    n = N
    o = np.arange(n)
    gens = []
    for g in range(perm.shape[0]):
        idx = np.argmax(perm[g], axis=0).astype(np.int64)  # y[o] = x[idx[o]]
        # rotation candidate: idx[o] = (o - r) % n ; idx[n+o] = n + (o-r)%n
        r = int((-idx[0]) % n)
        rot = np.concatenate([(o - r) % n, n + (o - r) % n])
        if np.array_equal(idx, rot):
            s = -r if r <= n // 2 else n - r
            gens.append((False, s))
            continue
        # reflection candidate: y[o] = xr[(o+r)%n per half] with
        # xr[t] = x[n + (-t)%n], xr[n+t] = x[(-t)%n]
        # => idx[o] = n + (-o-r)%n ; idx[n+o] = (-o-r)%n
        r = int(idx[0] - n) % n     # idx[0] = n + (-r)%n -> (-r)%n
        r = (-r) % n
        refl = np.concatenate([n + (-o - r) % n, (-o - r) % n])
        if np.array_equal(idx, refl):
            s = r if r <= n // 2 else r - n
            gens.append((True, s))
            continue
        raise NotImplementedError(f"perm[{g}] is not a dihedral rep matrix")
    for is_refl, s in gens:
        if is_refl:
            ok = -(HALO - 1) <= s <= HALO
        else:
            ok = -HALO <= s <= HALO
        if not ok:
            raise NotImplementedError(f"shift {s} exceeds halo {HALO}")
    return gens


def _build_program(gens):
    """Build + compile the SPMD Bass program (identical on all cores)."""
    rot = [(j, s) for j, (is_r, s) in enumerate(gens) if not is_r]
    refl = [(j, s) for j, (is_r, s) in enumerate(gens) if is_r]
    nblk = len(rot) + len(refl)

    nc = bacc.Bacc("TRN2", target_bir_lowering=False, debug=False,
                   num_devices=N_CORES)
    ax_d = nc.dram_tensor("ax", [128, NQ, 2, PH], _DT_IN,
                          kind="ExternalInput")
    ws_d = nc.dram_tensor("ws", [128, 32 * nblk], _DT_IN,
                          kind="ExternalInput")
    outr_d = nc.dram_tensor("outr", [128, NQ * L], _DT_OUT,
                            kind="ExternalOutput")

    CHUNKS = [2, 4, 4, 6]   # DMA chunk sizes along quads (small first
                            # chunk -> matmuls start earlier)
    with TileContext(nc) as tc:
        with (
            tc.tile_pool(name="arrp", bufs=1) as arrp,
            tc.tile_pool(name="wsp", bufs=1) as wsp,
            tc.tile_pool(name="stg", bufs=1) as stgp,
            tc.tile_pool(name="psum", bufs=1, space="PSUM") as psump,
        ):
            ws_sb = wsp.tile([128, 32 * nblk], _DT_IN)
            nc.sync.dma_start(out=ws_sb[:, :], in_=ws_d[:, :])
            ax_sb = arrp.tile([128, NQ, 2, PH], _DT_IN, name="ax_sb")
            c0 = 0
            for cq in CHUNKS:
                nc.sync.dma_start(out=ax_sb[:, c0:c0 + cq],
                                  in_=ax_d[:, c0:c0 + cq])
                c0 += cq

            # per quad: one matmul per generator per 32x32 PE quadrant
            # (batch element u -> SBUF rows 32u, PSUM partitions 32u);
            # the 4 quadrants run concurrently in the array.
            # (is_refl, weight block col, window param)
            mm_descs = []
            for k, (_, s) in enumerate(rot):
                mm_descs.append((False, 32 * k, s + HALO))
            for k, (_, s) in enumerate(refl):
                mm_descs.append((True, 32 * (len(rot) + k), s))
            ng = len(mm_descs)

            axt = ax_sb[:, :, :, :]
            pstride = axt.ap[0][0]      # free elems per partition

            def rhs_ap(p0, q, is_r, w):
                if not is_r:
                    return ax_sb[p0:p0 + 32, q, :, w:w + N]
                # reflection: swapped halves, backward o scan;
                # out (h, o) reads src[1-h, (HALO+200) - o - s]
                off = p0 * pstride + q * (2 * PH) + PH + (PH - HALO - w)
                return bass.AP(axt.tensor, off,
                               [[pstride, 32], [-PH, 2], [-1, N]])

            # pre-allocated rotating tiles (fewer tile instances -> fewer
            # semaphores -> shorter kernel-tail sem-reset storm)
            pstiles = [psump.tile([128, L], mybir.dt.float32,
                                  name=f"ps{i}") for i in range(4)]
            stgs = [stgp.tile([128, 2, L], _DT_OUT, name=f"stg{i}")
                    for i in range(4)]

            # ping-pong pairs of quads: quad B's matmul stream hides
            # quad A's PSUM drain between accumulation groups
            for qp in range(NQ // 2):
                pair = (2 * qp, 2 * qp + 1)
                pss = [pstiles[(2 * qp) % 4], pstiles[(2 * qp + 1) % 4]]
                for i, (is_r, wc, w) in enumerate(mm_descs):
                    for pi, q in enumerate(pair):
                        for u in range(4):
                            p0 = 32 * u
                            nc.tensor.matmul(
                                pss[pi][p0:p0 + 32, :],
                                ws_sb[p0:p0 + 32, wc:wc + 32],
                                rhs_ap(p0, q, is_r, w),
                                start=(i == 0), stop=(i == ng - 1),
                                tile_position=(p0, p0),
                            )
                stg = stgs[qp % 4]
                for pi, q in enumerate(pair):
                    if pi == 0:
                        nc.vector.tensor_copy(out=stg[:, 0], in_=pss[pi][:, :])
                    else:
                        nc.scalar.copy(out=stg[:, 1], in_=pss[pi][:, :])
                nc.sync.dma_start(out=outr_d[:, 2 * qp * L:(2 * qp + 2) * L],
                                  in_=stg[:, :, :])
    nc.compile()
    return nc


def _host_images(x, weight, gens):
    """Build per-core AX images and the packed weight image."""
    n = N
    rot = [(j, s) for j, (is_r, s) in enumerate(gens) if not is_r]
    refl = [(j, s) for j, (is_r, s) in enumerate(gens) if is_r]
    nblk = len(rot) + len(refl)

    pad_idx = (np.arange(PH) - HALO) % n
    xh = x.reshape(B, C, 2, n)[:, :, :, pad_idx]          # [B, C, 2, PH]

    ws = np.zeros((128, 32 * nblk), dtype=_NP_IN)
    for k, (j, _) in enumerate(rot + refl):
        for u in range(4):
            ws[32 * u:32 * (u + 1), 32 * k:32 * (k + 1)] = weight[j]

    def img(a, core):
        sl = a[core * BPC:(core + 1) * BPC]               # [64, C, 2, PH]
        out = np.empty((128, NQ, 2, PH), dtype=_NP_IN)
        for u in range(4):
            out[32 * u:32 * (u + 1)] = sl[u::4].transpose(1, 0, 2, 3)
        return np.ascontiguousarray(out)

    axs = [img(xh, c) for c in range(N_CORES)]
    return axs, ws


def _unscramble(outr):
    """outr[32*(b%4)+d, (b>>2)*L + o] -> out shard [BPC, D, L]."""
    r = outr.astype(np.float32).reshape(4, D, NQ, L)    # [b%4, d, q, o]
    r = r.transpose(2, 0, 1, 3)                         # [q, b%4, d, o]
    return np.ascontiguousarray(r.reshape(BPC, D, L))


def kernel(x, weight, perm, _trace=False):
    x = np.asarray(x, dtype=np.float32)
    weight = np.asarray(weight, dtype=np.float32)
    perm = np.asarray(perm, dtype=np.float32)

    gens = _derive_gens(perm)
    key = tuple(gens)
    if key not in _cache:
        _cache[key] = _build_program(gens)
    nc = _cache[key]

    axs, ws = _host_images(x, weight, gens)
    in_maps = [{"ax": axs[c], "ws": ws} for c in range(N_CORES)]
    res = run_bass_kernel_spmd(nc, in_maps, core_ids=list(range(N_CORES)),
                               trace=_trace)
    out = np.concatenate([_unscramble(res.results[c]["outr"])
                          for c in range(N_CORES)], axis=0)
    if _trace:
        kernel.last_exec_time_ns = res.exec_time_ns
        kernel.last_results = res
    return out
